# revision 11
# baseline (speedup 1.0000x reference)
"""Longformer local attention on 8 Trainium2 NeuronCores.

Problem: query [S=4096, B=2, E=768], H=12 heads, D=64, attention window 512
(one-sided W=256). QKV projections + banded attention, softmax over a
3W-key window per W-query chunk.

Sharding: batch (2) x head-groups (4) -> 8 cores. Each core computes the QKV
projection for its batch restricted to its 3 heads (192 of 768 output
channels per projection) over the full sequence, then banded attention for
those heads. No cross-core communication needed.

Per-core dataflow (all matmuls in float32r = full-rate fp32):
  - host passes x.T [768, 4096] (feature-major) so the contraction dim is on
    partitions without any on-chip transpose of x.
  - qkvT [576, 4096] = W_all.T @ x.T + b, computed in 5 M-tiles; the 576
    output channels are ordered [q0 q1 | k0 k1 | q2 v0 | k2 v1 | v2] so that
    head pairs (0,1) occupy partition ranges 0:64 / 64:128 of the same SBUF
    tiles (enables packed K=64 matmuls via tile_position rows).
  - v is PE-transposed to token-major [4096, 3*65] with a ones column per
    head (the ones column makes the attention matmul also produce the
    softmax denominator as row 64).
  - per chunk c (256 queries) and head: scores.T [768 kt, 256 q] via 6
    K=64 matmuls (heads 0,1 packed via tile_position), exp on ACT, band
    corners masked by tril/triu multiplies on DVE, then out.T [65, 256]
    = v_aug.T @ exp accumulated over the 6 key tiles. The attn@V matmuls
    run one chunk behind QK so exp/mask latency hides under PE work.
  - output written unnormalized ([64 d + 1 denom] x 4096 per head);
    the host divides and transposes.
"""

import numpy as np

S = 4096
E = 768
H = 12
D = 64
W = 256  # one-sided window = query chunk size
NCH = S // W  # 16 chunks
EO = 576  # per-core projection output channels (3 heads x 64 x {q,k,v})
TT = 512  # projection t-tile width
NCORES = 8

_cache = {}


def _build_program(reps=1, phases=(1, 1, 1)):
    import concourse.mybir as mybir
    import concourse.tile as tile
    from concourse import bacc

    f32 = mybir.dt.float32
    f32r = mybir.dt.float32r
    AF = mybir.ActivationFunctionType
    MUL = mybir.AluOpType.mult

    nc = bacc.Bacc("TRN2", target_bir_lowering=False, num_devices=NCORES)

    xT_d = nc.declare_dram_parameter("xT", [E, S], f32, isOutput=False)
    W_d = nc.declare_dram_parameter("Wall", [E, EO], f32, isOutput=False)
    b_d = nc.declare_dram_parameter("ball", [EO, 1], f32, isOutput=False)
    mask_d = nc.declare_dram_parameter("masks", [128, 768], f32, isOutput=False)
    id_d = nc.declare_dram_parameter("ident", [128, 128], f32, isOutput=False)
    o_d = nc.declare_dram_parameter("o", [195, S], f32, isOutput=True)

    # projection M-tiles: (eo_start, size)
    MT = [(0, 128), (128, 128), (256, 128), (384, 128), (512, 64)]
    KT = E // 128  # 6 contraction tiles

    with tile.TileContext(nc) as tc:
        with (
            tc.tile_pool(name="const", bufs=1) as cp,
            tc.tile_pool(name="qkv", bufs=1) as qp,
            tc.tile_pool(name="vsb", bufs=1) as vp,
        ):
            # --- constants ---
            mstage = cp.tile([128, 768], f32, tag="mstage")
            nc.sync.dma_start(mstage[:], mask_d[:])
            mask_r = cp.tile([128, 768], f32r, tag="maskr")
            nc.vector.tensor_copy(mask_r[:], mstage[:])
            actwarm = cp.tile([128, 1], f32, tag="actwarm")
            nc.scalar.activation(actwarm[:], mstage[:, 0:1], AF.Exp)

            istage = cp.tile([128, 128], f32, tag="istage")
            nc.sync.dma_start(istage[:], id_d[:])
            ident = cp.tile([128, 128], f32r, tag="ident")
            nc.vector.tensor_copy(ident[:], istage[:])

            bias_sb = cp.tile([128, len(MT)], f32, tag="bias")
            for m, (eo0, ms) in enumerate(MT):
                nc.sync.dma_start(bias_sb[:ms, m : m + 1], b_d[eo0 : eo0 + ms, :])

            Wr = []
            with tc.tile_pool(name="wstage", bufs=2) as wsp:
                for k in range(KT):
                    wst = wsp.tile([128, EO], f32, tag="wst", name="wst")
                    nc.sync.dma_start(wst[:], W_d[k * 128 : (k + 1) * 128, :])
                    wr = cp.tile([128, EO], f32r, tag=f"wr{k}", name=f"wr{k}")
                    nc.vector.tensor_copy(wr[:], wst[:])
                    Wr.append(wr)

            for _rep in range(reps):
                # persistent activations
                qkvT = [
                    qp.tile([ms, S], f32r, tag=f"qkvT{m}", name=f"qkvT{m}")
                    for m, (_, ms) in enumerate(MT)
                ]
                # v token-major, 3 heads x (64 + ones col) per 128-token block
                v_sb = vp.tile([128, 32 * 196], f32r, tag="v", name="v_sb")

                # --- QKV projection ---
                if phases[0]:
                    with (
                        tc.tile_pool(name="xin", bufs=12) as xip,
                        tc.tile_pool(name="xr", bufs=2) as xrp,
                        tc.tile_pool(name="pps", bufs=4, space="PSUM") as ppsp,
                    ):
                        for t in range(S // TT):
                            xr = []
                            for k in range(KT):
                                xin = xip.tile([128, TT], f32, tag="xin")
                                nc.sync.dma_start(
                                    xin[:],
                                    xT_d[
                                        k * 128 : (k + 1) * 128, t * TT : (t + 1) * TT
                                    ],
                                )
                                xrk = xrp.tile([128, TT], f32r, tag=f"xr{k}")
                                nc.vector.tensor_copy(xrk[:], xin[:])
                                xr.append(xrk)
                            for m, (eo0, ms) in enumerate(MT):
                                ps = ppsp.tile([ms, TT], f32, tag="pps")
                                for k in range(KT):
                                    nc.tensor.matmul(
                                        ps[:],
                                        Wr[k][:, eo0 : eo0 + ms],
                                        xr[k][:],
                                        start=(k == 0),
                                        stop=(k == KT - 1),
                                    )
                                nc.vector.tensor_scalar_add(
                                    qkvT[m][:, t * TT : (t + 1) * TT],
                                    ps[:],
                                    bias_sb[:ms, m : m + 1],
                                )

                # --- V transpose to token-major (+ ones columns) ---
                if phases[1]:
                    with tc.tile_pool(name="vtr", bufs=3, space="PSUM") as vtrp:
                        for tb in range(32):
                            c0 = 196 * tb
                            tcol = slice(tb * 128, (tb + 1) * 128)
                            pt = vtrp.tile([128, 192], f32r, tag="vtr", name="vtr")
                            # out = src.T @ I[:, sel] selects the v rows of the
                            # transposed block while keeping base partition 0
                            for h, (src, r0) in enumerate(
                                ((qkvT[2], 64), (qkvT[3], 64), (qkvT[4], 0))
                            ):
                                nc.tensor.matmul(
                                    pt[:, 64 * h : 64 * h + 64],
                                    src[:, tcol],
                                    ident[0 : src.shape[0], r0 : r0 + 64],
                                    is_transpose=True,
                                    start=True,
                                    stop=True,
                                )
                            dst = v_sb[:, c0 : c0 + 195].rearrange(
                                "p (h x) -> p h x", h=3
                            )[:, :, 0:64]
                            nc.vector.tensor_copy(
                                dst, pt[:].rearrange("p (h x) -> p h x", h=3)
                            )
                        # ones columns (denominator rows), one op per head
                        for h in range(3):
                            ones_ap = v_sb[:].rearrange("p (t c) -> p c t", t=32)[
                                :, 65 * h + 64, :
                            ]
                            nc.vector.tensor_scalar(
                                ones_ap,
                                mask_r[:, 0:32],
                                0.0,
                                1.0,
                                mybir.AluOpType.mult,
                                mybir.AluOpType.add,
                            )

                if not phases[2]:
                    continue

                # --- banded attention ---
                HEADS = [
                    (qkvT[0], qkvT[1], 0),
                    (qkvT[0], qkvT[1], 64),
                    (qkvT[2], qkvT[3], 0),
                ]
                TRIL = mask_r[:, 0:128]
                TRIU = mask_r[:, 128:256]
                TRILZ = mask_r[:, 256:512]
                ZTRIU = mask_r[:, 512:768]

                def jlist_of(c):
                    if c == 0:
                        return [2, 3, 4, 5]
                    if c == NCH - 1:
                        return [0, 1, 2, 3]
                    return [0, 1, 2, 3, 4, 5]

                with (
                    tc.tile_pool(name="ps0", bufs=2, space="PSUM") as ps0p,
                    tc.tile_pool(name="ps1", bufs=2, space="PSUM") as ps1p,
                    tc.tile_pool(name="ps2", bufs=2, space="PSUM") as ps2p,
                    tc.tile_pool(name="av", bufs=2, space="PSUM") as avp,
                    tc.tile_pool(name="ex", bufs=20) as exp_p,
                    tc.tile_pool(name="osb", bufs=3) as osbp,
                ):

                    def emit_qk(c):
                        """QK^T + exp + mask for chunk c; returns extiles."""
                        jl = jlist_of(c)
                        pairs = [tuple(jl[i : i + 2]) for i in range(0, len(jl), 2)]
                        qcol = slice(c * W, (c + 1) * W)
                        extiles = {}
                        for pi, jp in enumerate(pairs):
                            pstiles = []
                            for h in range(3):
                                pool = (ps0p, ps1p, ps2p)[h]
                                pstiles.append(
                                    pool.tile(
                                        [128, 512], f32, tag=f"ps{h}", name=f"ps{h}"
                                    )
                                )
                            for idx, j in enumerate(jp):
                                kcol = slice(
                                    (2 * c - 2 + j) * 128, (2 * c - 1 + j) * 128
                                )
                                for h, (qt, kt, pb) in enumerate(HEADS):
                                    nc.tensor.matmul(
                                        pstiles[h][:, idx * 256 : idx * 256 + 256],
                                        kt[pb : pb + 64, kcol],
                                        qt[pb : pb + 64, qcol],
                                        start=(idx == 0),
                                        stop=True,
                                        tile_position=(pb, 0),
                                    )
                            for h in range(3):
                                ex = exp_p.tile([128, 512], f32r, tag="ex", name="ex")
                                ps = pstiles[h]
                                nc.scalar.activation(ex[:], ps[:], AF.Exp)
                                for idx, j in enumerate(jp):
                                    lo = idx * 256
                                    if j == 0:
                                        nc.vector.tensor_tensor(
                                            ex[:, lo : lo + 256],
                                            ex[:, lo : lo + 256],
                                            TRILZ,
                                            MUL,
                                        )
                                    elif j == 1:
                                        nc.vector.tensor_tensor(
                                            ex[:, lo + 128 : lo + 256],
                                            ex[:, lo + 128 : lo + 256],
                                            TRIL,
                                            MUL,
                                        )
                                    elif j == 4:
                                        nc.vector.tensor_tensor(
                                            ex[:, lo : lo + 128],
                                            ex[:, lo : lo + 128],
                                            TRIU,
                                            MUL,
                                        )
                                    elif j == 5:
                                        nc.vector.tensor_tensor(
                                            ex[:, lo : lo + 256],
                                            ex[:, lo : lo + 256],
                                            ZTRIU,
                                            MUL,
                                        )
                                extiles[(h, pi)] = (ex, jp)
                        return extiles

                    def emit_av(c, extiles):
                        jl = jlist_of(c)
                        pairs = [tuple(jl[i : i + 2]) for i in range(0, len(jl), 2)]
                        qcol = slice(c * W, (c + 1) * W)
                        av01 = avp.tile([65, 512], f32, tag="av", name="av01")
                        av2 = avp.tile([65, 512], f32, tag="av", name="av2")
                        for h in range(3):
                            dst = av01 if h < 2 else av2
                            col0 = 256 * h if h < 2 else 0
                            first = True
                            for pi, jp in enumerate(pairs):
                                ex, _ = extiles[(h, pi)]
                                for idx, j in enumerate(jp):
                                    vb = 2 * c - 2 + j
                                    nc.tensor.matmul(
                                        dst[:, col0 : col0 + 256],
                                        v_sb[
                                            :,
                                            196 * vb + 65 * h : 196 * vb + 65 * h + 65,
                                        ],
                                        ex[:, idx * 256 : idx * 256 + 256],
                                        start=(h % 2 == 0 and first),
                                        stop=(
                                            pi == len(pairs) - 1 and idx == len(jp) - 1
                                        ),
                                    )
                                    first = False
                        o01 = osbp.tile([65, 512], f32, tag="o01", name="o01")
                        nc.vector.tensor_copy(o01[:], av01[:])
                        o2 = osbp.tile([65, 256], f32, tag="o2", name="o2")
                        nc.vector.tensor_copy(o2[:], av2[:, 0:256])
                        nc.sync.dma_start(o_d[0:65, qcol], o01[:, 0:256])
                        nc.sync.dma_start(o_d[65:130, qcol], o01[:, 256:512])
                        nc.sync.dma_start(o_d[130:195, qcol], o2[:])

                    pending = None  # (c, extiles) awaiting attn@V
                    for c in range(NCH + 1):
                        if c < NCH:
                            ext = emit_qk(c)
                            if pending is not None:
                                emit_av(*pending)
                            pending = (c, ext)
                        else:
                            emit_av(*pending)

    nc.compile()
    return nc


def _prep_inputs(query, Wq, bq, Wk, bk, Wv, bv):
    """Build the 8 per-core input maps."""
    scale = 1.0 / np.sqrt(np.float32(D))
    Wq_s = (Wq * scale).astype(np.float32)
    bq_s = (bq * scale).astype(np.float32)

    tril = (np.arange(128)[:, None] >= np.arange(128)[None, :]).astype(np.float32)
    triu = (np.arange(128)[:, None] <= np.arange(128)[None, :]).astype(np.float32)
    zero = np.zeros((128, 128), np.float32)
    masks = np.concatenate([tril, triu, tril, zero, zero, triu], axis=1)
    ident = np.eye(128, dtype=np.float32)

    xTs = [np.ascontiguousarray(query[:, b, :].T) for b in range(query.shape[1])]

    ORDER = [
        ("q", 0), ("q", 1), ("k", 0), ("k", 1), ("q", 2),
        ("v", 0), ("k", 2), ("v", 1), ("v", 2),
    ]
    Wmap = {"q": Wq_s, "k": Wk, "v": Wv}
    bmap = {"q": bq_s, "k": bk, "v": bv}

    in_maps = []
    for core in range(NCORES):
        b = core // 4
        g = core % 4
        wcols, bcols = [], []
        for typ, h in ORDER:
            hh = 3 * g + h
            wcols.append(Wmap[typ][:, 64 * hh : 64 * hh + 64])
            bcols.append(bmap[typ][64 * hh : 64 * hh + 64])
        W_all = np.ascontiguousarray(np.concatenate(wcols, axis=1), dtype=np.float32)
        b_all = np.concatenate(bcols)[:, None].astype(np.float32)
        in_maps.append(
            {
                "xT": xTs[b],
                "Wall": W_all,
                "ball": b_all,
                "masks": masks,
                "ident": ident,
            }
        )
    return in_maps


def _run(in_maps, trace=False):
    from concourse.bass_utils import run_bass_kernel_spmd

    if "nc" not in _cache:
        _cache["nc"] = _build_program()
    return run_bass_kernel_spmd(_cache["nc"], in_maps, list(range(NCORES)), trace=trace)


def kernel(query, Wq, bq, Wk, bk, Wv, bv):
    in_maps = _prep_inputs(query, Wq, bq, Wk, bk, Wv, bv)
    res = _run(in_maps)
    B = query.shape[1]
    out = np.empty((S, B, E), dtype=np.float32)
    for core in range(NCORES):
        b = core // 4
        g = core % 4
        o = res.results[core]["o"]
        for h in range(3):
            blk = o[65 * h : 65 * h + 64]
            den = o[65 * h + 64 : 65 * h + 65]
            col = 192 * g + 64 * h
            out[:, b, col : col + 64] = (blk / den).T
    return out


# revision 14
# speedup vs baseline: 1.3282x; 1.3282x over previous
"""Longformer local attention on 8 Trainium2 NeuronCores.

Problem: query [S=4096, B=2, E=768], H=12 heads, D=64, attention window 512
(one-sided W=256). QKV projections + banded attention, softmax over a
3W-key window per W-query chunk.

Sharding: batch (2) x head-groups (4) -> 8 cores. Each core computes the QKV
projection for its batch restricted to its 3 heads (192 of 768 output
channels per projection) over the full sequence, then banded attention for
those heads. No cross-core communication needed.

Per-core dataflow (all matmuls in float32r = full-rate fp32):
  - host passes x.T [768, 4096] (feature-major) so the contraction dim is on
    partitions without any on-chip transpose of x.
  - qkvT [576, 4096] = W_all.T @ x.T + b, computed in 5 M-tiles; the 576
    output channels are ordered [q0 q1 | k0 k1 | q2 v0 | k2 v1 | v2] so that
    head pairs (0,1) occupy partition ranges 0:64 / 64:128 of the same SBUF
    tiles (enables packed K=64 matmuls via tile_position rows).
  - v is PE-transposed to token-major [4096, 3*65] with a ones column per
    head (the ones column makes the attention matmul also produce the
    softmax denominator as row 64).
  - per chunk c (256 queries) and head: scores.T [768 kt, 256 q] via 6
    K=64 matmuls (heads 0,1 packed via tile_position), exp on ACT, band
    corners masked by tril/triu multiplies on DVE, then out.T [65, 256]
    = v_aug.T @ exp accumulated over the 6 key tiles. The attn@V matmuls
    run one chunk behind QK so exp/mask latency hides under PE work.
  - output written unnormalized ([64 d + 1 denom] x 4096 per head);
    the host divides and transposes.
"""

import numpy as np

S = 4096
E = 768
H = 12
D = 64
W = 256  # one-sided window = query chunk size
NCH = S // W  # 16 chunks
EO = 576  # per-core projection output channels (3 heads x 64 x {q,k,v})
TT = 512  # projection t-tile width
NCORES = 8

_cache = {}


def _build_program(reps=1, phases=(1, 1, 1)):
    import concourse.mybir as mybir
    import concourse.tile as tile
    from concourse import bacc

    f32 = mybir.dt.float32
    f32r = mybir.dt.float32r
    AF = mybir.ActivationFunctionType
    MUL = mybir.AluOpType.mult

    nc = bacc.Bacc("TRN2", target_bir_lowering=False, num_devices=NCORES)

    xT_d = nc.declare_dram_parameter("xT", [E, S], f32, isOutput=False)
    W_d = nc.declare_dram_parameter("Wall", [E, EO], f32, isOutput=False)
    b_d = nc.declare_dram_parameter("ball", [EO, 1], f32, isOutput=False)
    mask_d = nc.declare_dram_parameter("masks", [128, 768], f32, isOutput=False)
    id_d = nc.declare_dram_parameter("ident", [128, 128], f32, isOutput=False)
    o_d = nc.declare_dram_parameter("o", [195, S], f32, isOutput=True)

    # projection M-tiles: (eo_start, size)
    MT = [(0, 128), (128, 128), (256, 128), (384, 128), (512, 64)]
    KT = E // 128  # 6 contraction tiles

    with tile.TileContext(nc) as tc:
        with (
            tc.tile_pool(name="const", bufs=1) as cp,
            tc.tile_pool(name="qkv", bufs=1) as qp,
            tc.tile_pool(name="vsb", bufs=1) as vp,
        ):
            # --- constants ---
            mask_r = cp.tile([128, 768], f32r, tag="maskr")
            ident = cp.tile([128, 128], f32r, tag="ident")
            bias_sb = cp.tile([128, len(MT)], f32, tag="bias")
            for m, (eo0, ms) in enumerate(MT):
                nc.sync.dma_start(bias_sb[:ms, m : m + 1], b_d[eo0 : eo0 + ms, :])

            Wr = []
            with tc.tile_pool(name="wstage", bufs=2) as wsp:
                mstage = wsp.tile([128, 768], f32, tag="mstage")
                nc.sync.dma_start(mstage[:], mask_d[:])
                nc.vector.tensor_copy(mask_r[:], mstage[:])
                actwarm = cp.tile([128, 1], f32, tag="actwarm")
                nc.scalar.activation(actwarm[:], mstage[:, 0:1], AF.Exp)
                istage = wsp.tile([128, 128], f32, tag="istage")
                nc.sync.dma_start(istage[:], id_d[:])
                nc.vector.tensor_copy(ident[:], istage[:])
                for k in range(KT):
                    wst = wsp.tile([128, EO], f32, tag="wst", name="wst")
                    nc.sync.dma_start(wst[:], W_d[k * 128 : (k + 1) * 128, :])
                    wr = cp.tile([128, EO], f32r, tag=f"wr{k}", name=f"wr{k}")
                    nc.vector.tensor_copy(wr[:], wst[:])
                    Wr.append(wr)

            for _rep in range(reps):
                # persistent activations
                qkvT = [
                    qp.tile([ms, S], f32r, tag=f"qkvT{m}", name=f"qkvT{m}")
                    for m, (_, ms) in enumerate(MT)
                ]
                # v token-major, 3 heads x (64 + ones col) per 128-token block
                v_sb = vp.tile([128, 32 * 196], f32r, tag="v", name="v_sb")

                HEADS = [
                    (qkvT[0], qkvT[1], 0),
                    (qkvT[0], qkvT[1], 64),
                    (qkvT[2], qkvT[3], 0),
                ]
                TRIL = mask_r[:, 0:128]
                TRIU = mask_r[:, 128:256]
                TRILZ = mask_r[:, 256:512]
                ZTRIU = mask_r[:, 512:768]

                def jlist_of(c):
                    if c == 0:
                        return [2, 3, 4, 5]
                    if c == NCH - 1:
                        return [0, 1, 2, 3]
                    return [0, 1, 2, 3, 4, 5]

                with (
                    tc.tile_pool(name="xin", bufs=12) as xip,
                    tc.tile_pool(name="xr", bufs=1) as xrp,
                    tc.tile_pool(name="pps", bufs=2, space="PSUM") as ppsp,
                    tc.tile_pool(name="vtr", bufs=1, space="PSUM") as vtrp,
                    tc.tile_pool(name="ps0", bufs=2, space="PSUM") as ps0p,
                    tc.tile_pool(name="ps1", bufs=1, space="PSUM") as ps1p,
                    tc.tile_pool(name="ps2", bufs=1, space="PSUM") as ps2p,
                    tc.tile_pool(name="av", bufs=1, space="PSUM") as avp,
                    tc.tile_pool(name="ex", bufs=12) as exp_p,
                    tc.tile_pool(name="osb", bufs=2) as osbp,
                ):

                    def emit_proj(t):
                        xr = []
                        for k in range(KT):
                            xin = xip.tile([128, TT], f32, tag="xin", name="xin")
                            nc.sync.dma_start(
                                xin[:],
                                xT_d[k * 128 : (k + 1) * 128, t * TT : (t + 1) * TT],
                            )
                            xrk = xrp.tile([128, TT], f32r, tag=f"xr{k}", name=f"xr{k}")
                            nc.vector.tensor_copy(xrk[:], xin[:])
                            xr.append(xrk)
                        for m, (eo0, ms) in enumerate(MT):
                            ps = ppsp.tile([ms, TT], f32, tag="pps", name="pps")
                            for k in range(KT):
                                nc.tensor.matmul(
                                    ps[:],
                                    Wr[k][:, eo0 : eo0 + ms],
                                    xr[k][:],
                                    start=(k == 0),
                                    stop=(k == KT - 1),
                                )
                            nc.vector.tensor_scalar_add(
                                qkvT[m][:, t * TT : (t + 1) * TT],
                                ps[:],
                                bias_sb[:ms, m : m + 1],
                            )

                    def emit_vtrans(tb):
                        c0 = 196 * tb
                        tcol = slice(tb * 128, (tb + 1) * 128)
                        pt = vtrp.tile([128, 192], f32r, tag="vtr", name="vtr")
                        # out = src.T @ I[:, sel] selects the v rows of the
                        # transposed block while keeping base partition 0
                        for h, (src, r0) in enumerate(
                            ((qkvT[2], 64), (qkvT[3], 64), (qkvT[4], 0))
                        ):
                            nc.tensor.matmul(
                                pt[:, 64 * h : 64 * h + 64],
                                src[:, tcol],
                                ident[0 : src.shape[0], r0 : r0 + 64],
                                is_transpose=True,
                                start=True,
                                stop=True,
                            )
                        dst = v_sb[:, c0 : c0 + 195].rearrange(
                            "p (h x) -> p h x", h=3
                        )[:, :, 0:64]
                        nc.vector.tensor_copy(
                            dst, pt[:].rearrange("p (h x) -> p h x", h=3)
                        )

                    def emit_ones(t):
                        # denominator ones columns for blocks 4t..4t+3
                        for h in range(3):
                            ones_ap = v_sb[:].rearrange("p (t c) -> p c t", t=32)[
                                :, 65 * h + 64, 4 * t : 4 * t + 4
                            ]
                            nc.vector.tensor_scalar(
                                ones_ap,
                                mask_r[:, 0:4],
                                0.0,
                                1.0,
                                mybir.AluOpType.mult,
                                mybir.AluOpType.add,
                            )

                    def emit_qk(c):
                        """QK^T + exp + mask for chunk c; returns extiles."""
                        jl = jlist_of(c)
                        pairs = [tuple(jl[i : i + 2]) for i in range(0, len(jl), 2)]
                        qcol = slice(c * W, (c + 1) * W)
                        extiles = {}
                        for pi, jp in enumerate(pairs):
                            pstiles = []
                            for h in range(3):
                                pool = (ps0p, ps1p, ps2p)[h]
                                pstiles.append(
                                    pool.tile(
                                        [128, 512], f32, tag=f"ps{h}", name=f"ps{h}"
                                    )
                                )
                            for idx, j in enumerate(jp):
                                kcol = slice(
                                    (2 * c - 2 + j) * 128, (2 * c - 1 + j) * 128
                                )
                                for h, (qt, kt, pb) in enumerate(HEADS):
                                    nc.tensor.matmul(
                                        pstiles[h][:, idx * 256 : idx * 256 + 256],
                                        kt[pb : pb + 64, kcol],
                                        qt[pb : pb + 64, qcol],
                                        start=(idx == 0),
                                        stop=True,
                                        tile_position=(pb, 0),
                                    )
                            for h in range(3):
                                ex = exp_p.tile([128, 512], f32r, tag="ex", name="ex")
                                ps = pstiles[h]
                                nc.scalar.activation(ex[:], ps[:], AF.Exp)
                                for idx, j in enumerate(jp):
                                    lo = idx * 256
                                    if j == 0:
                                        nc.gpsimd.tensor_tensor(
                                            ex[:, lo : lo + 256],
                                            ex[:, lo : lo + 256],
                                            TRILZ,
                                            MUL,
                                        )
                                    elif j == 1:
                                        nc.gpsimd.tensor_tensor(
                                            ex[:, lo + 128 : lo + 256],
                                            ex[:, lo + 128 : lo + 256],
                                            TRIL,
                                            MUL,
                                        )
                                    elif j == 4:
                                        nc.gpsimd.tensor_tensor(
                                            ex[:, lo : lo + 128],
                                            ex[:, lo : lo + 128],
                                            TRIU,
                                            MUL,
                                        )
                                    elif j == 5:
                                        nc.gpsimd.tensor_tensor(
                                            ex[:, lo : lo + 256],
                                            ex[:, lo : lo + 256],
                                            ZTRIU,
                                            MUL,
                                        )
                                extiles[(h, pi)] = (ex, jp)
                        return extiles

                    def emit_av(c, extiles):
                        jl = jlist_of(c)
                        pairs = [tuple(jl[i : i + 2]) for i in range(0, len(jl), 2)]
                        qcol = slice(c * W, (c + 1) * W)
                        av01 = avp.tile([65, 512], f32, tag="av", name="av01")
                        av2 = avp.tile([65, 512], f32, tag="av", name="av2")
                        for h in range(3):
                            dst = av01 if h < 2 else av2
                            col0 = 256 * h if h < 2 else 0
                            first = True
                            for pi, jp in enumerate(pairs):
                                ex, _ = extiles[(h, pi)]
                                for idx, j in enumerate(jp):
                                    vb = 2 * c - 2 + j
                                    nc.tensor.matmul(
                                        dst[:, col0 : col0 + 256],
                                        v_sb[
                                            :,
                                            196 * vb + 65 * h : 196 * vb + 65 * h + 65,
                                        ],
                                        ex[:, idx * 256 : idx * 256 + 256],
                                        start=(h % 2 == 0 and first),
                                        stop=(
                                            pi == len(pairs) - 1 and idx == len(jp) - 1
                                        ),
                                    )
                                    first = False
                        o01 = osbp.tile([65, 512], f32, tag="o01", name="o01")
                        nc.vector.tensor_copy(o01[:], av01[:])
                        o2 = osbp.tile([65, 256], f32, tag="o2", name="o2")
                        nc.vector.tensor_copy(o2[:], av2[:, 0:256])
                        nc.sync.dma_start(o_d[0:65, qcol], o01[:, 0:256])
                        nc.sync.dma_start(o_d[65:130, qcol], o01[:, 256:512])
                        nc.sync.dma_start(o_d[130:195, qcol], o2[:])

                    # interleaved schedule: proj(t) -> vtrans 4t..4t+3 ->
                    # attention chunks c <= 2t (QK leads attn@V by one chunk)
                    pending = None
                    next_c = 0

                    def advance_attn(cmax):
                        nonlocal pending, next_c
                        while next_c <= min(cmax, NCH - 1):
                            ext = emit_qk(next_c)
                            if pending is not None:
                                emit_av(*pending)
                            pending = (next_c, ext)
                            next_c += 1

                    for t in range(S // TT):
                        if phases[0]:
                            emit_proj(t)
                        if phases[1]:
                            for tb in range(4 * t, 4 * t + 4):
                                emit_vtrans(tb)
                            emit_ones(t)
                        if phases[2]:
                            advance_attn(2 * t)
                    if phases[2]:
                        advance_attn(NCH - 1)
                        emit_av(*pending)

    nc.compile()
    return nc


def _prep_inputs(query, Wq, bq, Wk, bk, Wv, bv):
    """Build the 8 per-core input maps."""
    scale = 1.0 / np.sqrt(np.float32(D))
    Wq_s = (Wq * scale).astype(np.float32)
    bq_s = (bq * scale).astype(np.float32)

    tril = (np.arange(128)[:, None] >= np.arange(128)[None, :]).astype(np.float32)
    triu = (np.arange(128)[:, None] <= np.arange(128)[None, :]).astype(np.float32)
    zero = np.zeros((128, 128), np.float32)
    masks = np.concatenate([tril, triu, tril, zero, zero, triu], axis=1)
    ident = np.eye(128, dtype=np.float32)

    xTs = [np.ascontiguousarray(query[:, b, :].T) for b in range(query.shape[1])]

    ORDER = [
        ("q", 0), ("q", 1), ("k", 0), ("k", 1), ("q", 2),
        ("v", 0), ("k", 2), ("v", 1), ("v", 2),
    ]
    Wmap = {"q": Wq_s, "k": Wk, "v": Wv}
    bmap = {"q": bq_s, "k": bk, "v": bv}

    in_maps = []
    for core in range(NCORES):
        b = core // 4
        g = core % 4
        wcols, bcols = [], []
        for typ, h in ORDER:
            hh = 3 * g + h
            wcols.append(Wmap[typ][:, 64 * hh : 64 * hh + 64])
            bcols.append(bmap[typ][64 * hh : 64 * hh + 64])
        W_all = np.ascontiguousarray(np.concatenate(wcols, axis=1), dtype=np.float32)
        b_all = np.concatenate(bcols)[:, None].astype(np.float32)
        in_maps.append(
            {
                "xT": xTs[b],
                "Wall": W_all,
                "ball": b_all,
                "masks": masks,
                "ident": ident,
            }
        )
    return in_maps


def _run(in_maps, trace=False):
    from concourse.bass_utils import run_bass_kernel_spmd

    if "nc" not in _cache:
        _cache["nc"] = _build_program()
    return run_bass_kernel_spmd(_cache["nc"], in_maps, list(range(NCORES)), trace=trace)


def kernel(query, Wq, bq, Wk, bk, Wv, bv):
    in_maps = _prep_inputs(query, Wq, bq, Wk, bk, Wv, bv)
    res = _run(in_maps)
    B = query.shape[1]
    out = np.empty((S, B, E), dtype=np.float32)
    for core in range(NCORES):
        b = core // 4
        g = core % 4
        o = res.results[core]["o"]
        for h in range(3):
            blk = o[65 * h : 65 * h + 64]
            den = o[65 * h + 64 : 65 * h + 65]
            col = 192 * g + 64 * h
            out[:, b, col : col + 64] = (blk / den).T
    return out


# revision 16
# speedup vs baseline: 3.9059x; 2.9407x over previous
"""Longformer local attention on 8 Trainium2 NeuronCores.

Problem: query [S=4096, B=2, E=768], H=12 heads, D=64, attention window 512
(one-sided W=256). QKV projections + banded attention, softmax over a
3W-key window per W-query chunk.

Sharding: batch (2) x head-groups (4) -> 8 cores. Each core computes the QKV
projection for its batch restricted to its 3 heads (192 of 768 output
channels per projection) over the full sequence, then banded attention for
those heads. No cross-core communication needed.

Per-core dataflow (all matmuls in float32r = full-rate fp32):
  - host passes x.T [768, 4096] (feature-major) so the contraction dim is on
    partitions without any on-chip transpose of x.
  - qkvT [576, 4096] = W_all.T @ x.T + b, computed in 5 M-tiles; the 576
    output channels are ordered [q0 q1 | k0 k1 | q2 v0 | k2 v1 | v2] so that
    head pairs (0,1) occupy partition ranges 0:64 / 64:128 of the same SBUF
    tiles (enables packed K=64 matmuls via tile_position rows).
  - v is PE-transposed to token-major [4096, 3*65] with a ones column per
    head (the ones column makes the attention matmul also produce the
    softmax denominator as row 64).
  - per chunk c (256 queries) and head: scores.T [768 kt, 256 q] via 6
    K=64 matmuls (heads 0,1 packed via tile_position), exp on ACT, band
    corners masked by tril/triu multiplies on DVE, then out.T [65, 256]
    = v_aug.T @ exp accumulated over the 6 key tiles. The attn@V matmuls
    run one chunk behind QK so exp/mask latency hides under PE work.
  - output written unnormalized ([64 d + 1 denom] x 4096 per head);
    the host divides and transposes.
"""

import numpy as np

S = 4096
E = 768
H = 12
D = 64
W = 256  # one-sided window = query chunk size
NCH = S // W  # 16 chunks
EO = 576  # per-core projection output channels (3 heads x 64 x {q,k,v})
TT = 512  # projection t-tile width
NCORES = 8

_cache = {}


def _build_program(reps=1, phases=(1, 1, 1)):
    import concourse.mybir as mybir
    import concourse.tile as tile
    from concourse import bacc

    f32 = mybir.dt.float32
    f32r = mybir.dt.float32r
    AF = mybir.ActivationFunctionType
    MUL = mybir.AluOpType.mult

    nc = bacc.Bacc("TRN2", target_bir_lowering=False, num_devices=NCORES)

    xT_d = nc.declare_dram_parameter("xT", [E, S], f32, isOutput=False)
    W_d = nc.declare_dram_parameter("Wall", [E, EO], f32, isOutput=False)
    b_d = nc.declare_dram_parameter("ball", [EO, 1], f32, isOutput=False)
    mask_d = nc.declare_dram_parameter("masks", [128, 768], f32, isOutput=False)
    id_d = nc.declare_dram_parameter("ident", [128, 128], f32, isOutput=False)
    o_d = nc.declare_dram_parameter("o", [195, S], f32, isOutput=True)

    # projection M-tiles: (eo_start, size)
    MT = [(0, 128), (128, 128), (256, 128), (384, 128), (512, 64)]
    KT = E // 128  # 6 contraction tiles

    with tile.TileContext(nc) as tc:
        with (
            tc.tile_pool(name="const", bufs=1) as cp,
            tc.tile_pool(name="qkv", bufs=1) as qp,
            tc.tile_pool(name="vsb", bufs=1) as vp,
        ):
            # --- constants --- (tiles here; loads emitted inside rep 0 so
            # the first projection t-tile's x loads lead the DMA queue)
            mask_r = cp.tile([128, 768], f32r, tag="maskr")
            ident = cp.tile([128, 128], f32r, tag="ident")
            bias_sb = cp.tile([128, len(MT)], f32, tag="bias")
            Wr = [
                cp.tile([128, EO], f32r, tag=f"wr{k}", name=f"wr{k}")
                for k in range(KT)
            ]

            def emit_consts():
                with tc.tile_pool(name="wstage", bufs=2) as wsp:
                    for k in range(KT):
                        wst = wsp.tile([128, EO], f32, tag="wst", name="wst")
                        nc.sync.dma_start(wst[:], W_d[k * 128 : (k + 1) * 128, :])
                        nc.vector.tensor_copy(Wr[k][:], wst[:])
                    for m, (eo0, ms) in enumerate(MT):
                        nc.sync.dma_start(
                            bias_sb[:ms, m : m + 1], b_d[eo0 : eo0 + ms, :]
                        )
                    mstage = wsp.tile([128, 768], f32, tag="mstage")
                    nc.sync.dma_start(mstage[:], mask_d[:])
                    nc.vector.tensor_copy(mask_r[:], mstage[:])
                    actwarm = cp.tile([128, 1], f32, tag="actwarm")
                    nc.scalar.activation(actwarm[:], mstage[:, 0:1], AF.Exp)
                    istage = wsp.tile([128, 128], f32, tag="istage")
                    nc.sync.dma_start(istage[:], id_d[:])
                    nc.vector.tensor_copy(ident[:], istage[:])

            for _rep in range(reps):
                # persistent activations
                qkvT = [
                    qp.tile([ms, S], f32r, tag=f"qkvT{m}", name=f"qkvT{m}")
                    for m, (_, ms) in enumerate(MT)
                ]
                # v token-major, 3 heads x (64 + ones col) per 128-token block
                v_sb = vp.tile([128, 32 * 196], f32r, tag="v", name="v_sb")

                HEADS = [
                    (qkvT[0], qkvT[1], 0),
                    (qkvT[0], qkvT[1], 64),
                    (qkvT[2], qkvT[3], 0),
                ]
                TRIL = mask_r[:, 0:128]
                TRIU = mask_r[:, 128:256]
                TRILZ = mask_r[:, 256:512]
                ZTRIU = mask_r[:, 512:768]

                def jlist_of(c):
                    if c == 0:
                        return [2, 3, 4, 5]
                    if c == NCH - 1:
                        return [0, 1, 2, 3]
                    return [0, 1, 2, 3, 4, 5]

                with (
                    tc.tile_pool(name="xin", bufs=9) as xip,
                    tc.tile_pool(name="xr", bufs=1) as xrp,
                    tc.tile_pool(name="pps", bufs=2, space="PSUM") as ppsp,
                    tc.tile_pool(name="vtr", bufs=1, space="PSUM") as vtrp,
                    tc.tile_pool(name="ps0", bufs=2, space="PSUM") as ps0p,
                    tc.tile_pool(name="ps1", bufs=1, space="PSUM") as ps1p,
                    tc.tile_pool(name="ps2", bufs=1, space="PSUM") as ps2p,
                    tc.tile_pool(name="av", bufs=1, space="PSUM") as avp,
                    tc.tile_pool(name="ex", bufs=16) as exp_p,
                    tc.tile_pool(name="osb", bufs=2) as osbp,
                ):

                    def load_xr(t):
                        xr = []
                        for k in range(KT):
                            xin = xip.tile([128, TT], f32, tag="xin", name="xin")
                            nc.sync.dma_start(
                                xin[:],
                                xT_d[k * 128 : (k + 1) * 128, t * TT : (t + 1) * TT],
                            )
                            xrk = xrp.tile([128, TT], f32r, tag=f"xr{k}", name=f"xr{k}")
                            nc.vector.tensor_copy(xrk[:], xin[:])
                            xr.append(xrk)
                        return xr

                    def emit_proj(t, xr=None):
                        if xr is None:
                            xr = load_xr(t)
                        for m, (eo0, ms) in enumerate(MT):
                            ps = ppsp.tile([ms, TT], f32, tag="pps", name="pps")
                            for k in range(KT):
                                nc.tensor.matmul(
                                    ps[:],
                                    Wr[k][:, eo0 : eo0 + ms],
                                    xr[k][:],
                                    start=(k == 0),
                                    stop=(k == KT - 1),
                                )
                            nc.vector.tensor_scalar_add(
                                qkvT[m][:, t * TT : (t + 1) * TT],
                                ps[:],
                                bias_sb[:ms, m : m + 1],
                            )

                    def emit_vtrans(tb):
                        c0 = 196 * tb
                        tcol = slice(tb * 128, (tb + 1) * 128)
                        pt = vtrp.tile([128, 192], f32r, tag="vtr", name="vtr")
                        # out = src.T @ I[:, sel] selects the v rows of the
                        # transposed block while keeping base partition 0
                        for h, (src, r0) in enumerate(
                            ((qkvT[2], 64), (qkvT[3], 64), (qkvT[4], 0))
                        ):
                            nc.tensor.matmul(
                                pt[:, 64 * h : 64 * h + 64],
                                src[:, tcol],
                                ident[0 : src.shape[0], r0 : r0 + 64],
                                is_transpose=True,
                                start=True,
                                stop=True,
                            )
                        dst = v_sb[:, c0 : c0 + 195].rearrange(
                            "p (h x) -> p h x", h=3
                        )[:, :, 0:64]
                        nc.vector.tensor_copy(
                            dst, pt[:].rearrange("p (h x) -> p h x", h=3)
                        )

                    def emit_ones(t):
                        # denominator ones columns for blocks 4t..4t+3
                        for h in range(3):
                            ones_ap = v_sb[:].rearrange("p (t c) -> p c t", t=32)[
                                :, 65 * h + 64, 4 * t : 4 * t + 4
                            ]
                            nc.vector.tensor_scalar(
                                ones_ap,
                                mask_r[:, 0:4],
                                0.0,
                                1.0,
                                mybir.AluOpType.mult,
                                mybir.AluOpType.add,
                            )

                    def emit_qk(c):
                        """QK^T + exp + mask for chunk c; returns extiles."""
                        jl = jlist_of(c)
                        pairs = [tuple(jl[i : i + 2]) for i in range(0, len(jl), 2)]
                        qcol = slice(c * W, (c + 1) * W)
                        extiles = {}
                        for pi, jp in enumerate(pairs):
                            pstiles = []
                            for h in range(3):
                                pool = (ps0p, ps1p, ps2p)[h]
                                pstiles.append(
                                    pool.tile(
                                        [128, 512], f32, tag=f"ps{h}", name=f"ps{h}"
                                    )
                                )
                            for idx, j in enumerate(jp):
                                kcol = slice(
                                    (2 * c - 2 + j) * 128, (2 * c - 1 + j) * 128
                                )
                                for h, (qt, kt, pb) in enumerate(HEADS):
                                    nc.tensor.matmul(
                                        pstiles[h][:, idx * 256 : idx * 256 + 256],
                                        kt[pb : pb + 64, kcol],
                                        qt[pb : pb + 64, qcol],
                                        start=(idx == 0),
                                        stop=True,
                                        tile_position=(pb, 0),
                                    )
                            for h in range(3):
                                ex = exp_p.tile([128, 512], f32r, tag="ex", name="ex")
                                ps = pstiles[h]
                                nc.scalar.activation(ex[:], ps[:], AF.Exp)
                                for idx, j in enumerate(jp):
                                    lo = idx * 256
                                    if j == 0:
                                        nc.gpsimd.tensor_tensor(
                                            ex[:, lo : lo + 256],
                                            ex[:, lo : lo + 256],
                                            TRILZ,
                                            MUL,
                                        )
                                    elif j == 1:
                                        nc.gpsimd.tensor_tensor(
                                            ex[:, lo + 128 : lo + 256],
                                            ex[:, lo + 128 : lo + 256],
                                            TRIL,
                                            MUL,
                                        )
                                    elif j == 4:
                                        nc.gpsimd.tensor_tensor(
                                            ex[:, lo : lo + 128],
                                            ex[:, lo : lo + 128],
                                            TRIU,
                                            MUL,
                                        )
                                    elif j == 5:
                                        nc.gpsimd.tensor_tensor(
                                            ex[:, lo : lo + 256],
                                            ex[:, lo : lo + 256],
                                            ZTRIU,
                                            MUL,
                                        )
                                extiles[(h, pi)] = (ex, jp)
                        return extiles

                    def emit_av(c, extiles):
                        jl = jlist_of(c)
                        pairs = [tuple(jl[i : i + 2]) for i in range(0, len(jl), 2)]
                        qcol = slice(c * W, (c + 1) * W)
                        av01 = avp.tile([65, 512], f32, tag="av", name="av01")
                        av2 = avp.tile([65, 512], f32, tag="av", name="av2")
                        for h in range(3):
                            dst = av01 if h < 2 else av2
                            col0 = 256 * h if h < 2 else 0
                            first = True
                            for pi, jp in enumerate(pairs):
                                ex, _ = extiles[(h, pi)]
                                for idx, j in enumerate(jp):
                                    vb = 2 * c - 2 + j
                                    nc.tensor.matmul(
                                        dst[:, col0 : col0 + 256],
                                        v_sb[
                                            :,
                                            196 * vb + 65 * h : 196 * vb + 65 * h + 65,
                                        ],
                                        ex[:, idx * 256 : idx * 256 + 256],
                                        start=(h % 2 == 0 and first),
                                        stop=(
                                            pi == len(pairs) - 1 and idx == len(jp) - 1
                                        ),
                                    )
                                    first = False
                        ot = osbp.tile([65, 768], f32, tag="ot", name="ot")
                        nc.vector.tensor_copy(ot[:, 0:512], av01[:])
                        nc.vector.tensor_copy(ot[:, 512:768], av2[:, 0:256])
                        nc.sync.dma_start(
                            o_d[:, qcol].rearrange("(h r) q -> r h q", h=3),
                            ot[:].rearrange("r (h q) -> r h q", h=3),
                        )

                    # interleaved schedule: proj(t) -> vtrans 4t..4t+3 ->
                    # attention chunks c <= 2t (QK leads attn@V by one chunk)
                    pending = None
                    next_c = 0

                    def advance_attn(cmax):
                        nonlocal pending, next_c
                        while next_c <= min(cmax, NCH - 1):
                            ext = emit_qk(next_c)
                            if pending is not None:
                                emit_av(*pending)
                            pending = (next_c, ext)
                            next_c += 1

                    xr0 = None
                    if phases[0]:
                        xr0 = load_xr(0)
                    if _rep == 0:
                        emit_consts()
                    for t in range(S // TT):
                        if phases[0]:
                            emit_proj(t, xr0 if t == 0 else None)
                        if phases[1]:
                            for tb in range(4 * t, 4 * t + 4):
                                emit_vtrans(tb)
                            emit_ones(t)
                        if phases[2]:
                            advance_attn(2 * t)
                    if phases[2]:
                        advance_attn(NCH - 1)
                        emit_av(*pending)

    nc.compile()
    return nc


def _prep_inputs(query, Wq, bq, Wk, bk, Wv, bv):
    """Build the 8 per-core input maps."""
    scale = 1.0 / np.sqrt(np.float32(D))
    Wq_s = (Wq * scale).astype(np.float32)
    bq_s = (bq * scale).astype(np.float32)

    tril = (np.arange(128)[:, None] >= np.arange(128)[None, :]).astype(np.float32)
    triu = (np.arange(128)[:, None] <= np.arange(128)[None, :]).astype(np.float32)
    zero = np.zeros((128, 128), np.float32)
    masks = np.concatenate([tril, triu, tril, zero, zero, triu], axis=1)
    ident = np.eye(128, dtype=np.float32)

    xTs = [np.ascontiguousarray(query[:, b, :].T) for b in range(query.shape[1])]

    ORDER = [
        ("q", 0), ("q", 1), ("k", 0), ("k", 1), ("q", 2),
        ("v", 0), ("k", 2), ("v", 1), ("v", 2),
    ]
    Wmap = {"q": Wq_s, "k": Wk, "v": Wv}
    bmap = {"q": bq_s, "k": bk, "v": bv}

    in_maps = []
    for core in range(NCORES):
        b = core // 4
        g = core % 4
        wcols, bcols = [], []
        for typ, h in ORDER:
            hh = 3 * g + h
            wcols.append(Wmap[typ][:, 64 * hh : 64 * hh + 64])
            bcols.append(bmap[typ][64 * hh : 64 * hh + 64])
        W_all = np.ascontiguousarray(np.concatenate(wcols, axis=1), dtype=np.float32)
        b_all = np.concatenate(bcols)[:, None].astype(np.float32)
        in_maps.append(
            {
                "xT": xTs[b],
                "Wall": W_all,
                "ball": b_all,
                "masks": masks,
                "ident": ident,
            }
        )
    return in_maps


def _run(in_maps, trace=False):
    from concourse.bass_utils import run_bass_kernel_spmd

    if "nc" not in _cache:
        _cache["nc"] = _build_program()
    return run_bass_kernel_spmd(_cache["nc"], in_maps, list(range(NCORES)), trace=trace)


def kernel(query, Wq, bq, Wk, bk, Wv, bv):
    in_maps = _prep_inputs(query, Wq, bq, Wk, bk, Wv, bv)
    res = _run(in_maps)
    B = query.shape[1]
    out = np.empty((S, B, E), dtype=np.float32)
    for core in range(NCORES):
        b = core // 4
        g = core % 4
        o = res.results[core]["o"]
        for h in range(3):
            blk = o[65 * h : 65 * h + 64]
            den = o[65 * h + 64 : 65 * h + 65]
            col = 192 * g + 64 * h
            out[:, b, col : col + 64] = (blk / den).T
    return out
